# revision 30
# baseline (speedup 1.0000x reference)
"""LIF spiking-neuron scan (SimpleSNN) Trainium2 Bass kernel.

Reference semantics (per sample b, neuron n, over T timesteps):
    mem = mem * 0.9 + x[t]
    spike[t] = (mem >= 1.5)
    mem = mem * (1 - spike[t])

Full inputs [256, 200, 1024] f32 are sharded batch-wise over 8 NeuronCores
(32 samples/core; the time recurrence is per-sample so no cross-core comms).

Host-side, each core's shard [32, 200, 1024] is permuted to a
partition-major layout [128, 200, 256] with partition p = k*32 + b
(k = n // 256, b = sample), so every chunk DMA is a single dense 3-D
transfer carrying one completion semaphore.

Per-core device design:
  - The recurrence is rewritten over the PRE-reset membrane w:
        w_t = select(w_{t-1} < 1.5, w_{t-1}, 0) * 0.9 + x_t
        spike_t = (w_t >= 1.5)
    which is bit-identical to the reference (same two f32 roundings per
    step) and needs only ONE fused custom-DVE op per step (5 ALU stages
    of the DVE's 8-stage pipeline). The w history is materialized in the
    chunk tile, so the whole sequential chain is 200 back-to-back Vector
    engine instructions at ~[128, 256] each.
  - T=200 steps split into ragged chunks (geometric ramp, small tail).
    Per chunk: one HWDGE DMA load of x [128, tc, 256] (SP ring), tc
    fused LIF-step ops (DVE), one batched Sign activation over the w
    chunk on the Scalar engine (sign(w + nextafter(-1.5, 0)) is +1 u8
    exactly iff w >= 1.5; the host maps ==1 -> 1.0f).
  - Spikes stay resident in SBUF (50 KiB/partition u8) and are written
    by four large deferred DMA stores (ACT ring) scheduled behind the x
    loads, because HBM reads and writes share the ~360 GB/s per-core
    budget and interleaved stores starve the loads below the chain rate.
  - Measured: 110.8 us/core (vs 801.4 us baseline); the 200-op serial
    DVE chain (~425 ns + 35 ns issue per step) is the critical path,
    with ~12 us of fixed startup (engine barriers + table loads + first
    chunk load) and ~6 us of tail (last Sign + store + final barrier).
"""

from contextlib import ExitStack

import numpy as np

B, T, N = 256, 200, 1024
NCORES = 8
BL = B // NCORES  # 32 samples per core
DECAY = 0.9
TH = 1.5
P128 = 128
FREE = 256  # free-dim size of the state tile
NK = N // FREE  # 4 n-blocks; partition p = k*32 + b
# Ragged chunking. The x loads (131 kB/step) at the ~360 GB/s per-core
# HBM cap run only ~20% faster than the serial DVE chain (~0.46
# us/step), so the loader builds slack slowly: start with a small chunk
# (chain starts ~3 us after the first bytes land) and grow
# geometrically. Small last chunk keeps the tail (final spike pass +
# store after the chain ends) short.
# PAIRED=True uses the hand-written two-step fused DVE op (half the
# serial-chain instruction count); False falls back to the proven
# one-step op. Flip this single flag to revert.
PAIRED = True
# All chunk sizes even: the fused DVE op processes time-step PAIRS.
CHUNKS = [6, 8, 10, 12, 16, 18, 22, 24, 24, 24, 18, 10, 4, 4]
assert sum(CHUNKS) == T
TCMAX = max(CHUNKS)
# Spikes for all 200 steps stay resident in SBUF (T*FREE u8 = 50 KiB per
# partition) and are stored in a few large deferred DMAs, scheduled so
# the writes drain mostly after the loads finish: HBM read+write share
# the ~360 GB/s per-core budget, and interleaved per-chunk stores were
# measured to slow the loads below the chain rate (pipeline stalls).
# Each store group gets its OWN SBUF tile: a single shared tile made
# Tile's whole-tile WAR tracking stall later Sign ops behind earlier
# groups' store reads (measured 6.8 us chain stall). The shrinking tail
# chunks keep the post-chain work (last Sign + last store) tiny.
STORE_AFTER_CHUNK = {8: (0, 140), 10: (140, 182), 12: (182, 196), 13: (196, 200)}
for _c, (_a, _b) in STORE_AFTER_CHUNK.items():
    assert sum(CHUNKS[: _c + 1]) == _b

_CACHE = {}

_LIF_OP_NAME = "LIF_STEP_ANT"


def _lif_reference(in0, in1, s0, s1, imm2):
    return (
        np.where(in0 < np.float32(s0), in0, np.float32(0.0)) * np.float32(s1) + in1
    ).astype(np.float32)


def _register_lif_op():
    """Register the fused LIF-step custom DVE op:
        out = select(in0 < s0, in0, 0) * s1 + in1
    (in0 = previous membrane w, in1 = x_t, s0 = threshold, s1 = decay).
    Registration is the runtime equivalent of appending to dve_ops.OPS;
    uops_sha is computed from the same lower() used at compile time.
    """
    import concourse.dve_ops as dve_ops
    from concourse.dve_ops import DveOp
    from concourse.dve_spec import C0, C1, Spec, Src0, Src1, Zero, lower, select
    from concourse.dve_uop import DveOpSpec

    if _LIF_OP_NAME in dve_ops._SUB_OPCODE_FOR_NAME:
        for op in dve_ops.OPS:
            if op.name == _LIF_OP_NAME:
                return op
        raise RuntimeError("LIF op registered but not in OPS")

    body = select(Src0 < C0, Src0, Zero) * C1 + Src1
    spec = Spec(body=body, reference=_lif_reference)
    row = dve_ops._CUSTOM_DVE_ROW_BASE + len(dve_ops.OPS)
    shas = {}
    for ver in ("v3", "v4"):
        uops = lower(spec, ver=ver)
        shas[ver] = DveOpSpec(
            name=_LIF_OP_NAME, opcode=row, uops=uops, rd1_en=True
        ).sha(ver)
    op = DveOp(_LIF_OP_NAME, spec, subdim=False, uops_sha=shas)
    dve_ops.OPS.append(op)
    dve_ops._SUB_OPCODE_FOR_NAME[_LIF_OP_NAME] = row
    dve_ops.CUSTOM_DVE_SPECS[_LIF_OP_NAME] = spec
    return op


_LIF2_OP_NAME = "LIF_STEP2_ANT"


def _register_lif2_op():
    """Register a hand-written two-step fused LIF op. One instruction
    processes PAIRS of time steps: the element stream is (w1[i], w2[i])
    pairs over i (f-major, pair-minor APs on the normal tile layout):

        w1[i] = select(w0[i] < s0, w0[i], 0) * s1 + x1[i]   (uop A)
        w2[i] = select(w1[i] < s0, w1[i], 0) * s1 + x2[i]   (uop B)

    A computes w1 in ALU blocks 0-3 (the stock 1-step program) and
    bypasses it to the write port. B (one element = one cycle behind)
    leaves blocks 0-3 idle with block 3's out-flop WRITE DISABLED, so the
    flop still holds A's w1 when B's block-4 compare reads PREV_ALU_OUT
    one cycle later; B computes in blocks 4-7. Same ALU sequence per step
    as the 1-step op -> bit-identical results, but the serial chain is
    100 instructions of 512 elements instead of 200 of 256, halving the
    per-instruction fixed cost (~151 cycles) on the critical path.

    DveOp.compile() would re-lower the placeholder Spec, so the hand
    program is pre-seeded into dve_ops._COMPILE_CACHE for both DVE
    generations; dve_table_for_ops and _custom_dve hit the cache.
    """
    import concourse.dve_ops as dve_ops
    from concourse.dve_ops import DveOp
    from concourse.dve_spec import C0, C1, Spec, Src0, Src1, Zero, select
    from concourse.dve_uop import (
        AluInp,
        AluOp,
        DelayInp,
        DveOpSpec,
        InpSel,
        OutPath,
        OutSel,
        Trigger,
        UopConfig,
    )

    if _LIF2_OP_NAME in dve_ops._SUB_OPCODE_FOR_NAME:
        for op in dve_ops.OPS:
            if op.name == _LIF2_OP_NAME:
                return op
        raise RuntimeError("LIF2 op registered but not in OPS")

    def mk_uop(first_of_pair: bool, next_idx: int) -> UopConfig:
        u = UopConfig()
        # Input lanes mirror the stock 1-step lowering: lane k feeds
        # block 0's delay chain k-1.
        u.enable_input(InpSel.SRC_1, 1)  # chain0: w0 (only A consumes it)
        u.enable_input(InpSel.CONST_0, 2)  # chain1: threshold s0
        u.enable_input(InpSel.ZERO, 3)  # chain2: 0.0
        u.enable_input(InpSel.CONST_1, 4)  # chain3: decay s1
        u.enable_input(InpSel.SRC_0, 5)  # chain4: x for this step
        u.require_inp0 = 1
        u.require_inp1 = 1
        u.repeat_count = 1
        u.trigger = (Trigger.SRC_TENSOR_DONE, Trigger.COUNT, Trigger.NONE)
        u.next_uop = (0, next_idx, 0)
        dp = u.datapath_config
        for b in range(8):
            dp[b].pass_through_delay(0, 1, 2, 3, 4)
        if first_of_pair:
            # blocks 0-3: the stock step; 4-7: bypass w1 to the write port
            dp[0].enable_alu(AluOp.IS_LT, AluInp.PREV_DELAY_0, AluInp.PREV_DELAY_1)
            dp[1].enable_alu(AluOp.SELECT, AluInp.PREV_DELAY_2, AluInp.PREV_DELAY_0)
            dp[2].enable_alu(AluOp.MULTIPLY, AluInp.PREV_ALU_OUT, AluInp.PREV_DELAY_3)
            dp[3].enable_alu(AluOp.ADD, AluInp.PREV_ALU_OUT, AluInp.PREV_DELAY_4)
            for b in (4, 5, 6, 7):
                dp[b].pass_through_alu()
        else:
            # blocks 0-3 idle (block 3's flop retains A's w1); compute in 4-7.
            dp[4].enable_alu(AluOp.IS_LT, AluInp.PREV_ALU_OUT, AluInp.PREV_DELAY_1)
            # capture w1 for the select operand (chain1: threshold is dead now)
            dp[4].enable_delay_from_src(DelayInp.PREV_ALU_OUT, 1)
            dp[5].enable_alu(AluOp.SELECT, AluInp.PREV_DELAY_2, AluInp.PREV_DELAY_1)
            dp[6].enable_alu(AluOp.MULTIPLY, AluInp.PREV_ALU_OUT, AluInp.PREV_DELAY_3)
            dp[7].enable_alu(AluOp.ADD, AluInp.PREV_ALU_OUT, AluInp.PREV_DELAY_4)
        u.enable_output(OutSel.ALU_OUT, OutPath.WR0_LO)
        return u

    # uop[0] may not be a jump target: [A-entry, B, A-loop], B <-> A-loop.
    uops = [mk_uop(True, 1), mk_uop(False, 2), mk_uop(True, 1)]

    # Placeholder Spec: gives _custom_dve the right flags (reads Src1, no
    # C2/accum) and CoreSim a 1-step reference; the hand uops are what the
    # hardware runs.
    body = select(Src0 < C0, Src0, Zero) * C1 + Src1
    spec = Spec(body=body, reference=_lif_reference)
    row = dve_ops._CUSTOM_DVE_ROW_BASE + len(dve_ops.OPS)
    handspec = DveOpSpec(name=_LIF2_OP_NAME, opcode=row, uops=uops, rd1_en=True)
    shas = {ver: handspec.sha(ver) for ver in ("v3", "v4")}
    op = DveOp(_LIF2_OP_NAME, spec, subdim=False, uops_sha=shas)
    dve_ops.OPS.append(op)
    dve_ops._SUB_OPCODE_FOR_NAME[_LIF2_OP_NAME] = row
    dve_ops.CUSTOM_DVE_SPECS[_LIF2_OP_NAME] = spec
    for ver in ("v3", "v4"):
        dve_ops._COMPILE_CACHE[(_LIF2_OP_NAME, ver)] = handspec
    return op


def _build_bass(reps: int = 1):
    # reps > 1 repeats the whole pipeline on the same buffers (benchmarking
    # only — amortizes host dispatch overhead to expose the device time).
    import concourse.bacc as bacc
    import concourse.tile as tile
    from concourse import mybir

    lif_op = _register_lif2_op() if PAIRED else _register_lif_op()

    nc = bacc.Bacc(
        "TRN2",
        target_bir_lowering=False,
        debug=False,
        enable_asserts=False,
    )

    P = P128
    f32 = mybir.dt.float32

    u8 = mybir.dt.uint8
    x_d = nc.dram_tensor("x", [P, T, FREE], f32, kind="ExternalInput").ap()
    s_d = nc.dram_tensor("spk", [P, T, FREE], u8, kind="ExternalOutput").ap()

    with ExitStack() as ctx:
        tc = ctx.enter_context(tile.TileContext(nc))
        xp = ctx.enter_context(tc.tile_pool(name="xp", bufs=3))
        wp = ctx.enter_context(tc.tile_pool(name="wp", bufs=2))
        sp = ctx.enter_context(tc.tile_pool(name="sp", bufs=1))
        st = ctx.enter_context(tc.tile_pool(name="st", bufs=1))

        zero = st.tile([P, FREE], f32)
        nc.vector.memset(zero[:], 0.0)
        # Spike threshold as an ACT bias: sign(w + SPIKE_BIAS) is +1 exactly
        # when w >= TH (SPIKE_BIAS = nextafter(-TH, 0), so w == TH lands one
        # ulp above zero and w == TH - 1ulp lands exactly on zero -> sign 0).
        spike_bias = st.tile([P, 1], f32, tag="bias")
        nc.vector.memset(
            spike_bias[:], float(np.nextafter(np.float32(-TH), np.float32(0)))
        )

        # Spikes accumulate in one tile per store group (deferred DMAs).
        spk_group = {}  # chunk index whose completion triggers the store -> (tile, a, b)
        for _c, (a, b) in STORE_AFTER_CHUNK.items():
            spk_group[_c] = (
                sp.tile([P, b - a, FREE], u8, name=f"spk{_c}", tag=f"s{_c}"),
                a,
                b,
            )

        def group_of(t):
            for _c, (tile_, a, b) in spk_group.items():
                if a <= t < b:
                    return tile_, a, b
            raise AssertionError(t)

        wt_prev = None
        prev_tc = None
        for c, tcsz in enumerate(CHUNKS * reps):
            t0 = sum(CHUNKS[: c % len(CHUNKS)])
            xt = xp.tile([P, TCMAX, FREE], f32, tag="x")
            # Loads ride the SP HWDGE ring, stores the ACT HWDGE ring —
            # two independent DMA queues that overlap.
            nc.sync.dma_start(out=xt[:, :tcsz, :], in_=x_d[:, t0 : t0 + tcsz, :])

            wt = wp.tile([P, TCMAX, FREE], f32, tag="w")
            step = 2 if PAIRED else 1
            for j0 in range(0, tcsz, step):
                if c == 0 and j0 == 0:
                    w_in = zero[:]
                elif j0 == 0:
                    w_in = wt_prev[:, prev_tc - 1, :]
                else:
                    w_in = wt[:, j0 - 1, :]
                if PAIRED:
                    # Two fused steps per instruction; element stream is
                    # (w1[i], w2[i]) pairs, f-major. in0 = matching x pairs,
                    # in1 = w0 broadcast (read once per pair by uop A).
                    nc.vector._custom_dve(
                        lif_op,
                        out=wt[:, j0 : j0 + 2, :].rearrange("p t f -> p f t"),
                        in0=xt[:, j0 : j0 + 2, :].rearrange("p t f -> p f t"),
                        in1=w_in.to_broadcast((P, FREE, 2)),
                        s0=TH,
                        s1=DECAY,
                    )
                else:
                    # w_t = select(w_{t-1} < TH, w_{t-1}, 0) * DECAY + x_t
                    nc.vector._custom_dve(
                        lif_op,
                        out=wt[:, j0, :],
                        in0=w_in,
                        in1=xt[:, j0, :],
                        s0=TH,
                        s1=DECAY,
                    )
            wt_prev = wt
            prev_tc = tcsz

            # spikes as sign(w + SPIKE_BIAS) in {-1, 0, +1} stored u8 (the
            # host maps ==1 -> 1.0f). Runs on the otherwise-idle Scalar
            # engine; GpSimd's tensor_scalar measures ~18 cyc/elem and
            # serializes the kernel, ACT streams at 1 elem/cycle.
            gt, ga, gb = group_of(t0)
            assert t0 + tcsz <= gb, "chunk spans store groups"
            nc.scalar.activation(
                out=gt[:, t0 - ga : t0 - ga + tcsz, :].rearrange(
                    "p t f -> p (t f)"
                ),
                in_=wt[:, :tcsz, :].rearrange("p t f -> p (t f)"),
                func=mybir.ActivationFunctionType.Sign,
                bias=spike_bias[:],
            )
            if c % len(CHUNKS) in STORE_AFTER_CHUNK:
                a, b = STORE_AFTER_CHUNK[c % len(CHUNKS)]
                nc.scalar.dma_start(out=s_d[:, a:b, :], in_=gt[:])

    # Bacc lowering: splits multi-wait instructions into event-semaphore
    # chains (TRN2 allows at most one sync wait per instruction), register
    # allocation, DCE.
    nc.compile()
    return nc


def _get_nc():
    if "nc" not in _CACHE:
        _CACHE["nc"] = _build_bass()
    return _CACHE["nc"]


def _shard_input(inputs: np.ndarray, i: int) -> np.ndarray:
    # [32, 200, 1024] -> [32, 200, 4, 256] -> [4, 32, 200, 256] -> [128, 200, 256]
    xi = inputs[i * BL : (i + 1) * BL]
    xi = xi.reshape(BL, T, NK, FREE).transpose(2, 0, 1, 3)
    return np.ascontiguousarray(xi).reshape(P128, T, FREE)


def _unshard_output(spk: np.ndarray) -> np.ndarray:
    # [128, 200, 256] u8 -> [4, 32, 200, 256] -> [32, 200, 4, 256] -> [32, 200, 1024]
    s = spk.reshape(NK, BL, T, FREE).transpose(1, 2, 0, 3)
    return np.ascontiguousarray(s).reshape(BL, T, N)


def kernel(inputs: np.ndarray, trace: bool = False) -> np.ndarray:
    from concourse.bass_utils import run_bass_kernel_spmd

    inputs = np.ascontiguousarray(np.asarray(inputs, dtype=np.float32))
    assert inputs.shape == (B, T, N), inputs.shape

    nc = _get_nc()
    in_maps = [{"x": _shard_input(inputs, i)} for i in range(NCORES)]
    res = run_bass_kernel_spmd(
        nc, in_maps, core_ids=list(range(NCORES)), trace=trace
    )
    _CACHE["last_results"] = res
    out = np.concatenate(
        [_unshard_output(r["spk"]) for r in res.results], axis=0
    )
    # Device stores sign(w + SPIKE_BIAS) as u8: +1 (= spike) maps to 1,
    # 0 and -1 (however the f32->u8 conversion encodes it) map to not-1.
    return (out == 1).astype(np.float32)



# revision 31
# speedup vs baseline: 1.1286x; 1.1286x over previous
"""LIF spiking-neuron scan (SimpleSNN) Trainium2 Bass kernel.

Reference semantics (per sample b, neuron n, over T timesteps):
    mem = mem * 0.9 + x[t]
    spike[t] = (mem >= 1.5)
    mem = mem * (1 - spike[t])

Full inputs [256, 200, 1024] f32 are sharded batch-wise over 8 NeuronCores
(32 samples/core; the time recurrence is per-sample so no cross-core comms).

Host-side, each core's shard [32, 200, 1024] is permuted to a
partition-major layout [128, 200, 256] with partition p = k*32 + b
(k = n // 256, b = sample), so every chunk DMA is a single dense 3-D
transfer carrying one completion semaphore.

Per-core device design:
  - The recurrence is rewritten over the PRE-reset membrane w:
        w_t = select(w_{t-1} < 1.5, w_{t-1}, 0) * 0.9 + x_t
        spike_t = (w_t >= 1.5)
    which is bit-identical to the reference (same two f32 roundings per
    step) and needs only ONE fused custom-DVE op per step (5 ALU stages
    of the DVE's 8-stage pipeline). The w history is materialized in the
    chunk tile, so the whole sequential chain is 200 back-to-back Vector
    engine instructions at ~[128, 256] each.
  - T=200 steps split into ragged chunks (geometric ramp, small tail).
    Per chunk: one HWDGE DMA load of x [128, tc, 256] (SP ring), tc
    fused LIF-step ops (DVE), one batched Sign activation over the w
    chunk on the Scalar engine (sign(w + nextafter(-1.5, 0)) is +1 u8
    exactly iff w >= 1.5; the host maps ==1 -> 1.0f).
  - Spikes stay resident in SBUF (50 KiB/partition u8) and are written
    by four large deferred DMA stores (ACT ring) scheduled behind the x
    loads, because HBM reads and writes share the ~360 GB/s per-core
    budget and interleaved stores starve the loads below the chain rate.
  - Measured: 110.8 us/core (vs 801.4 us baseline); the 200-op serial
    DVE chain (~425 ns + 35 ns issue per step) is the critical path,
    with ~12 us of fixed startup (engine barriers + table loads + first
    chunk load) and ~6 us of tail (last Sign + store + final barrier).
"""

from contextlib import ExitStack

import numpy as np

B, T, N = 256, 200, 1024
NCORES = 8
BL = B // NCORES  # 32 samples per core
DECAY = 0.9
TH = 1.5
P128 = 128
FREE = 256  # free-dim size of the state tile
NK = N // FREE  # 4 n-blocks; partition p = k*32 + b
# Ragged chunking. The x loads (131 kB/step) at the ~360 GB/s per-core
# HBM cap run only ~20% faster than the serial DVE chain (~0.46
# us/step), so the loader builds slack slowly: start with a small chunk
# (chain starts ~3 us after the first bytes land) and grow
# geometrically. Small last chunk keeps the tail (final spike pass +
# store after the chain ends) short.
# PAIRED=True uses the hand-written two-step fused DVE op (half the
# serial-chain instruction count). It is bit-exact on hardware, but with
# the current tile layout the pair APs are strided (inner stride = FREE
# elements), which drops the DVE stream to ~1.7-2 cycles/element (1070ns
# vs 425+425ns measured) — a net loss. It would win (~20us) with a
# pair-interleaved SBUF layout [P, T/2, FREE, 2] making in0/out
# contiguous; that needs matching host-side shard/unshard permutes.
PAIRED = False
# All chunk sizes even: the fused DVE op processes time-step PAIRS.
CHUNKS = [6, 8, 10, 12, 16, 18, 22, 24, 24, 24, 18, 10, 4, 4]
assert sum(CHUNKS) == T
TCMAX = max(CHUNKS)
# Spikes for all 200 steps stay resident in SBUF (T*FREE u8 = 50 KiB per
# partition) and are stored in a few large deferred DMAs, scheduled so
# the writes drain mostly after the loads finish: HBM read+write share
# the ~360 GB/s per-core budget, and interleaved per-chunk stores were
# measured to slow the loads below the chain rate (pipeline stalls).
# Each store group gets its OWN SBUF tile: a single shared tile made
# Tile's whole-tile WAR tracking stall later Sign ops behind earlier
# groups' store reads (measured 6.8 us chain stall). The shrinking tail
# chunks keep the post-chain work (last Sign + last store) tiny.
STORE_AFTER_CHUNK = {8: (0, 140), 10: (140, 182), 12: (182, 196), 13: (196, 200)}
for _c, (_a, _b) in STORE_AFTER_CHUNK.items():
    assert sum(CHUNKS[: _c + 1]) == _b

_CACHE = {}

_LIF_OP_NAME = "LIF_STEP_ANT"


def _lif_reference(in0, in1, s0, s1, imm2):
    return (
        np.where(in0 < np.float32(s0), in0, np.float32(0.0)) * np.float32(s1) + in1
    ).astype(np.float32)


def _register_lif_op():
    """Register the fused LIF-step custom DVE op:
        out = select(in0 < s0, in0, 0) * s1 + in1
    (in0 = previous membrane w, in1 = x_t, s0 = threshold, s1 = decay).
    Registration is the runtime equivalent of appending to dve_ops.OPS;
    uops_sha is computed from the same lower() used at compile time.
    """
    import concourse.dve_ops as dve_ops
    from concourse.dve_ops import DveOp
    from concourse.dve_spec import C0, C1, Spec, Src0, Src1, Zero, lower, select
    from concourse.dve_uop import DveOpSpec

    if _LIF_OP_NAME in dve_ops._SUB_OPCODE_FOR_NAME:
        for op in dve_ops.OPS:
            if op.name == _LIF_OP_NAME:
                return op
        raise RuntimeError("LIF op registered but not in OPS")

    body = select(Src0 < C0, Src0, Zero) * C1 + Src1
    spec = Spec(body=body, reference=_lif_reference)
    row = dve_ops._CUSTOM_DVE_ROW_BASE + len(dve_ops.OPS)
    shas = {}
    for ver in ("v3", "v4"):
        uops = lower(spec, ver=ver)
        shas[ver] = DveOpSpec(
            name=_LIF_OP_NAME, opcode=row, uops=uops, rd1_en=True
        ).sha(ver)
    op = DveOp(_LIF_OP_NAME, spec, subdim=False, uops_sha=shas)
    dve_ops.OPS.append(op)
    dve_ops._SUB_OPCODE_FOR_NAME[_LIF_OP_NAME] = row
    dve_ops.CUSTOM_DVE_SPECS[_LIF_OP_NAME] = spec
    return op


_LIF2_OP_NAME = "LIF_STEP2_ANT"


def _register_lif2_op():
    """Register a hand-written two-step fused LIF op. One instruction
    processes PAIRS of time steps: the element stream is (w1[i], w2[i])
    pairs over i (f-major, pair-minor APs on the normal tile layout):

        w1[i] = select(w0[i] < s0, w0[i], 0) * s1 + x1[i]   (uop A)
        w2[i] = select(w1[i] < s0, w1[i], 0) * s1 + x2[i]   (uop B)

    A computes w1 in ALU blocks 0-3 (the stock 1-step program) and
    bypasses it to the write port. B (one element = one cycle behind)
    leaves blocks 0-3 idle with block 3's out-flop WRITE DISABLED, so the
    flop still holds A's w1 when B's block-4 compare reads PREV_ALU_OUT
    one cycle later; B computes in blocks 4-7. Same ALU sequence per step
    as the 1-step op -> bit-identical results, but the serial chain is
    100 instructions of 512 elements instead of 200 of 256, halving the
    per-instruction fixed cost (~151 cycles) on the critical path.

    DveOp.compile() would re-lower the placeholder Spec, so the hand
    program is pre-seeded into dve_ops._COMPILE_CACHE for both DVE
    generations; dve_table_for_ops and _custom_dve hit the cache.
    """
    import concourse.dve_ops as dve_ops
    from concourse.dve_ops import DveOp
    from concourse.dve_spec import C0, C1, Spec, Src0, Src1, Zero, select
    from concourse.dve_uop import (
        AluInp,
        AluOp,
        DelayInp,
        DveOpSpec,
        InpSel,
        OutPath,
        OutSel,
        Trigger,
        UopConfig,
    )

    if _LIF2_OP_NAME in dve_ops._SUB_OPCODE_FOR_NAME:
        for op in dve_ops.OPS:
            if op.name == _LIF2_OP_NAME:
                return op
        raise RuntimeError("LIF2 op registered but not in OPS")

    def mk_uop(first_of_pair: bool, next_idx: int) -> UopConfig:
        u = UopConfig()
        # Input lanes mirror the stock 1-step lowering: lane k feeds
        # block 0's delay chain k-1.
        u.enable_input(InpSel.SRC_1, 1)  # chain0: w0 (only A consumes it)
        u.enable_input(InpSel.CONST_0, 2)  # chain1: threshold s0
        u.enable_input(InpSel.ZERO, 3)  # chain2: 0.0
        u.enable_input(InpSel.CONST_1, 4)  # chain3: decay s1
        u.enable_input(InpSel.SRC_0, 5)  # chain4: x for this step
        u.require_inp0 = 1
        u.require_inp1 = 1
        u.repeat_count = 1
        u.trigger = (Trigger.SRC_TENSOR_DONE, Trigger.COUNT, Trigger.NONE)
        u.next_uop = (0, next_idx, 0)
        dp = u.datapath_config
        for b in range(8):
            dp[b].pass_through_delay(0, 1, 2, 3, 4)
        if first_of_pair:
            # blocks 0-3: the stock step; 4-7: bypass w1 to the write port
            dp[0].enable_alu(AluOp.IS_LT, AluInp.PREV_DELAY_0, AluInp.PREV_DELAY_1)
            dp[1].enable_alu(AluOp.SELECT, AluInp.PREV_DELAY_2, AluInp.PREV_DELAY_0)
            dp[2].enable_alu(AluOp.MULTIPLY, AluInp.PREV_ALU_OUT, AluInp.PREV_DELAY_3)
            dp[3].enable_alu(AluOp.ADD, AluInp.PREV_ALU_OUT, AluInp.PREV_DELAY_4)
            for b in (4, 5, 6, 7):
                dp[b].pass_through_alu()
        else:
            # blocks 0-3 idle (block 3's flop retains A's w1); compute in 4-7.
            dp[4].enable_alu(AluOp.IS_LT, AluInp.PREV_ALU_OUT, AluInp.PREV_DELAY_1)
            # capture w1 for the select operand (chain1: threshold is dead now)
            dp[4].enable_delay_from_src(DelayInp.PREV_ALU_OUT, 1)
            dp[5].enable_alu(AluOp.SELECT, AluInp.PREV_DELAY_2, AluInp.PREV_DELAY_1)
            dp[6].enable_alu(AluOp.MULTIPLY, AluInp.PREV_ALU_OUT, AluInp.PREV_DELAY_3)
            dp[7].enable_alu(AluOp.ADD, AluInp.PREV_ALU_OUT, AluInp.PREV_DELAY_4)
        u.enable_output(OutSel.ALU_OUT, OutPath.WR0_LO)
        return u

    # uop[0] may not be a jump target: [A-entry, B, A-loop], B <-> A-loop.
    uops = [mk_uop(True, 1), mk_uop(False, 2), mk_uop(True, 1)]

    # Placeholder Spec: gives _custom_dve the right flags (reads Src1, no
    # C2/accum) and CoreSim a 1-step reference; the hand uops are what the
    # hardware runs.
    body = select(Src0 < C0, Src0, Zero) * C1 + Src1
    spec = Spec(body=body, reference=_lif_reference)
    row = dve_ops._CUSTOM_DVE_ROW_BASE + len(dve_ops.OPS)
    handspec = DveOpSpec(name=_LIF2_OP_NAME, opcode=row, uops=uops, rd1_en=True)
    shas = {ver: handspec.sha(ver) for ver in ("v3", "v4")}
    op = DveOp(_LIF2_OP_NAME, spec, subdim=False, uops_sha=shas)
    dve_ops.OPS.append(op)
    dve_ops._SUB_OPCODE_FOR_NAME[_LIF2_OP_NAME] = row
    dve_ops.CUSTOM_DVE_SPECS[_LIF2_OP_NAME] = spec
    for ver in ("v3", "v4"):
        dve_ops._COMPILE_CACHE[(_LIF2_OP_NAME, ver)] = handspec
    return op


def _build_bass(reps: int = 1):
    # reps > 1 repeats the whole pipeline on the same buffers (benchmarking
    # only — amortizes host dispatch overhead to expose the device time).
    import concourse.bacc as bacc
    import concourse.tile as tile
    from concourse import mybir

    lif_op = _register_lif2_op() if PAIRED else _register_lif_op()

    nc = bacc.Bacc(
        "TRN2",
        target_bir_lowering=False,
        debug=False,
        enable_asserts=False,
    )

    P = P128
    f32 = mybir.dt.float32

    u8 = mybir.dt.uint8
    x_d = nc.dram_tensor("x", [P, T, FREE], f32, kind="ExternalInput").ap()
    s_d = nc.dram_tensor("spk", [P, T, FREE], u8, kind="ExternalOutput").ap()

    with ExitStack() as ctx:
        tc = ctx.enter_context(tile.TileContext(nc))
        xp = ctx.enter_context(tc.tile_pool(name="xp", bufs=3))
        wp = ctx.enter_context(tc.tile_pool(name="wp", bufs=2))
        sp = ctx.enter_context(tc.tile_pool(name="sp", bufs=1))
        st = ctx.enter_context(tc.tile_pool(name="st", bufs=1))

        zero = st.tile([P, FREE], f32)
        nc.vector.memset(zero[:], 0.0)
        # Spike threshold as an ACT bias: sign(w + SPIKE_BIAS) is +1 exactly
        # when w >= TH (SPIKE_BIAS = nextafter(-TH, 0), so w == TH lands one
        # ulp above zero and w == TH - 1ulp lands exactly on zero -> sign 0).
        spike_bias = st.tile([P, 1], f32, tag="bias")
        nc.vector.memset(
            spike_bias[:], float(np.nextafter(np.float32(-TH), np.float32(0)))
        )

        # Spikes accumulate in one tile per store group (deferred DMAs).
        spk_group = {}  # chunk index whose completion triggers the store -> (tile, a, b)
        for _c, (a, b) in STORE_AFTER_CHUNK.items():
            spk_group[_c] = (
                sp.tile([P, b - a, FREE], u8, name=f"spk{_c}", tag=f"s{_c}"),
                a,
                b,
            )

        def group_of(t):
            for _c, (tile_, a, b) in spk_group.items():
                if a <= t < b:
                    return tile_, a, b
            raise AssertionError(t)

        wt_prev = None
        prev_tc = None
        for c, tcsz in enumerate(CHUNKS * reps):
            t0 = sum(CHUNKS[: c % len(CHUNKS)])
            xt = xp.tile([P, TCMAX, FREE], f32, tag="x")
            # Loads ride the SP HWDGE ring, stores the ACT HWDGE ring —
            # two independent DMA queues that overlap.
            nc.sync.dma_start(out=xt[:, :tcsz, :], in_=x_d[:, t0 : t0 + tcsz, :])

            wt = wp.tile([P, TCMAX, FREE], f32, tag="w")
            step = 2 if PAIRED else 1
            for j0 in range(0, tcsz, step):
                if c == 0 and j0 == 0:
                    w_in = zero[:]
                elif j0 == 0:
                    w_in = wt_prev[:, prev_tc - 1, :]
                else:
                    w_in = wt[:, j0 - 1, :]
                if PAIRED:
                    # Two fused steps per instruction; element stream is
                    # (w1[i], w2[i]) pairs, f-major. in0 = matching x pairs,
                    # in1 = w0 broadcast (read once per pair by uop A).
                    nc.vector._custom_dve(
                        lif_op,
                        out=wt[:, j0 : j0 + 2, :].rearrange("p t f -> p f t"),
                        in0=xt[:, j0 : j0 + 2, :].rearrange("p t f -> p f t"),
                        in1=w_in.to_broadcast((P, FREE, 2)),
                        s0=TH,
                        s1=DECAY,
                    )
                else:
                    # w_t = select(w_{t-1} < TH, w_{t-1}, 0) * DECAY + x_t
                    nc.vector._custom_dve(
                        lif_op,
                        out=wt[:, j0, :],
                        in0=w_in,
                        in1=xt[:, j0, :],
                        s0=TH,
                        s1=DECAY,
                    )
            wt_prev = wt
            prev_tc = tcsz

            # spikes as sign(w + SPIKE_BIAS) in {-1, 0, +1} stored u8 (the
            # host maps ==1 -> 1.0f). Runs on the otherwise-idle Scalar
            # engine; GpSimd's tensor_scalar measures ~18 cyc/elem and
            # serializes the kernel, ACT streams at 1 elem/cycle.
            gt, ga, gb = group_of(t0)
            assert t0 + tcsz <= gb, "chunk spans store groups"
            nc.scalar.activation(
                out=gt[:, t0 - ga : t0 - ga + tcsz, :].rearrange(
                    "p t f -> p (t f)"
                ),
                in_=wt[:, :tcsz, :].rearrange("p t f -> p (t f)"),
                func=mybir.ActivationFunctionType.Sign,
                bias=spike_bias[:],
            )
            if c % len(CHUNKS) in STORE_AFTER_CHUNK:
                a, b = STORE_AFTER_CHUNK[c % len(CHUNKS)]
                nc.scalar.dma_start(out=s_d[:, a:b, :], in_=gt[:])

    # Bacc lowering: splits multi-wait instructions into event-semaphore
    # chains (TRN2 allows at most one sync wait per instruction), register
    # allocation, DCE.
    nc.compile()
    return nc


def _get_nc():
    if "nc" not in _CACHE:
        _CACHE["nc"] = _build_bass()
    return _CACHE["nc"]


def _shard_input(inputs: np.ndarray, i: int) -> np.ndarray:
    # [32, 200, 1024] -> [32, 200, 4, 256] -> [4, 32, 200, 256] -> [128, 200, 256]
    xi = inputs[i * BL : (i + 1) * BL]
    xi = xi.reshape(BL, T, NK, FREE).transpose(2, 0, 1, 3)
    return np.ascontiguousarray(xi).reshape(P128, T, FREE)


def _unshard_output(spk: np.ndarray) -> np.ndarray:
    # [128, 200, 256] u8 -> [4, 32, 200, 256] -> [32, 200, 4, 256] -> [32, 200, 1024]
    s = spk.reshape(NK, BL, T, FREE).transpose(1, 2, 0, 3)
    return np.ascontiguousarray(s).reshape(BL, T, N)


def kernel(inputs: np.ndarray, trace: bool = False) -> np.ndarray:
    from concourse.bass_utils import run_bass_kernel_spmd

    inputs = np.ascontiguousarray(np.asarray(inputs, dtype=np.float32))
    assert inputs.shape == (B, T, N), inputs.shape

    nc = _get_nc()
    in_maps = [{"x": _shard_input(inputs, i)} for i in range(NCORES)]
    res = run_bass_kernel_spmd(
        nc, in_maps, core_ids=list(range(NCORES)), trace=trace
    )
    _CACHE["last_results"] = res
    out = np.concatenate(
        [_unshard_output(r["spk"]) for r in res.results], axis=0
    )
    # Device stores sign(w + SPIKE_BIAS) as u8: +1 (= spike) maps to 1,
    # 0 and -1 (however the f32->u8 conversion encodes it) map to not-1.
    return (out == 1).astype(np.float32)



# revision 36
# speedup vs baseline: 1.1740x; 1.0402x over previous
"""LIF spiking-neuron scan (SimpleSNN) Trainium2 Bass kernel.

Reference semantics (per sample b, neuron n, over T timesteps):
    mem = mem * 0.9 + x[t]
    spike[t] = (mem >= 1.5)
    mem = mem * (1 - spike[t])

Full inputs [256, 200, 1024] f32 are sharded batch-wise over 8 NeuronCores
(32 samples/core; the time recurrence is per-sample so no cross-core comms).

Host-side, each core's shard [32, 200, 1024] is permuted to a
partition-major layout [128, 200, 256] with partition p = k*32 + b
(k = n // 256, b = sample), so every chunk DMA is a single dense 3-D
transfer carrying one completion semaphore.

Per-core device design:
  - The recurrence is rewritten over the PRE-reset membrane w:
        w_t = select(w_{t-1} < 1.5, w_{t-1}, 0) * 0.9 + x_t
        spike_t = (w_t >= 1.5)
    which is bit-identical to the reference (same two f32 roundings per
    step) and needs only ONE fused custom-DVE op per step (5 ALU stages
    of the DVE's 8-stage pipeline). The w history is materialized in the
    chunk tile, so the whole sequential chain is 200 back-to-back Vector
    engine instructions at ~[128, 256] each.
  - T=200 steps split into ragged chunks (geometric ramp, small tail).
    Per chunk: one HWDGE DMA load of x [128, tc, 256] (SP ring), tc
    fused LIF-step ops (DVE), one batched Sign activation over the w
    chunk on the Scalar engine (sign(w + nextafter(-1.5, 0)) is +1 u8
    exactly iff w >= 1.5; the host maps ==1 -> 1.0f).
  - Spikes stay resident in SBUF (50 KiB/partition u8) and are written
    by four large deferred DMA stores (ACT ring) scheduled behind the x
    loads, because HBM reads and writes share the ~360 GB/s per-core
    budget and interleaved stores starve the loads below the chain rate.
  - Measured: 110.8 us/core (vs 801.4 us baseline); the 200-op serial
    DVE chain (~425 ns + 35 ns issue per step) is the critical path,
    with ~12 us of fixed startup (engine barriers + table loads + first
    chunk load) and ~6 us of tail (last Sign + store + final barrier).
"""

from contextlib import ExitStack

import numpy as np

B, T, N = 256, 200, 1024
NCORES = 8
BL = B // NCORES  # 32 samples per core
DECAY = 0.9
TH = 1.5
P128 = 128
FREE = 256  # free-dim size of the state tile
NK = N // FREE  # 4 n-blocks; partition p = k*32 + b
# Ragged chunking. The x loads (131 kB/step) at the ~360 GB/s per-core
# HBM cap run only ~20% faster than the serial DVE chain (~0.46
# us/step), so the loader builds slack slowly: start with a small chunk
# (chain starts ~3 us after the first bytes land) and grow
# geometrically. Small last chunk keeps the tail (final spike pass +
# store after the chain ends) short.
# The serial chain uses the hand-written two-step fused DVE op (100
# instructions instead of 200, halving the ~151-cycle per-instruction
# fixed cost on the critical path). The x/w/spike layouts are
# PAIR-INTERLEAVED [P, T/2, FREE, 2] so the fused op's in0/out streams
# are unit-stride: with the plain layout the pair APs jumped 1 KiB at
# the innermost step and the SBUF port's 8-byte fetch granularity
# dropped the stream to ~2 cycles/element (1070 ns/op measured vs the
# ~690 ns contiguous cost).
PAIRED = True
# All chunk sizes even: the fused DVE op processes time-step PAIRS.
CHUNKS = [6, 8, 10, 12, 16, 18, 22, 24, 24, 24, 18, 10, 4, 4]
assert sum(CHUNKS) == T
TCMAX = max(CHUNKS)
# Spikes for all 200 steps stay resident in SBUF (T*FREE u8 = 50 KiB per
# partition) and are stored in a few large deferred DMAs, scheduled so
# the writes drain mostly after the loads finish: HBM read+write share
# the ~360 GB/s per-core budget, and interleaved per-chunk stores were
# measured to slow the loads below the chain rate (pipeline stalls).
# Each store group gets its OWN SBUF tile: a single shared tile made
# Tile's whole-tile WAR tracking stall later Sign ops behind earlier
# groups' store reads (measured 6.8 us chain stall). The shrinking tail
# chunks keep the post-chain work (last Sign + last store) tiny.
STORE_AFTER_CHUNK = {8: (0, 140), 10: (140, 182), 12: (182, 196), 13: (196, 200)}
for _c, (_a, _b) in STORE_AFTER_CHUNK.items():
    assert sum(CHUNKS[: _c + 1]) == _b

_CACHE = {}

_LIF_OP_NAME = "LIF_STEP_ANT"


def _lif_reference(in0, in1, s0, s1, imm2):
    return (
        np.where(in0 < np.float32(s0), in0, np.float32(0.0)) * np.float32(s1) + in1
    ).astype(np.float32)


def _register_lif_op():
    """Register the fused LIF-step custom DVE op:
        out = select(in0 < s0, in0, 0) * s1 + in1
    (in0 = previous membrane w, in1 = x_t, s0 = threshold, s1 = decay).
    Registration is the runtime equivalent of appending to dve_ops.OPS;
    uops_sha is computed from the same lower() used at compile time.
    """
    import concourse.dve_ops as dve_ops
    from concourse.dve_ops import DveOp
    from concourse.dve_spec import C0, C1, Spec, Src0, Src1, Zero, lower, select
    from concourse.dve_uop import DveOpSpec

    if _LIF_OP_NAME in dve_ops._SUB_OPCODE_FOR_NAME:
        for op in dve_ops.OPS:
            if op.name == _LIF_OP_NAME:
                return op
        raise RuntimeError("LIF op registered but not in OPS")

    body = select(Src0 < C0, Src0, Zero) * C1 + Src1
    spec = Spec(body=body, reference=_lif_reference)
    row = dve_ops._CUSTOM_DVE_ROW_BASE + len(dve_ops.OPS)
    shas = {}
    for ver in ("v3", "v4"):
        uops = lower(spec, ver=ver)
        shas[ver] = DveOpSpec(
            name=_LIF_OP_NAME, opcode=row, uops=uops, rd1_en=True
        ).sha(ver)
    op = DveOp(_LIF_OP_NAME, spec, subdim=False, uops_sha=shas)
    dve_ops.OPS.append(op)
    dve_ops._SUB_OPCODE_FOR_NAME[_LIF_OP_NAME] = row
    dve_ops.CUSTOM_DVE_SPECS[_LIF_OP_NAME] = spec
    return op


_LIF2_OP_NAME = "LIF_STEP2_ANT"


def _register_lif2_op():
    """Register a hand-written two-step fused LIF op. One instruction
    processes PAIRS of time steps: the element stream is (w1[i], w2[i])
    pairs over i (f-major, pair-minor APs on the normal tile layout):

        w1[i] = select(w0[i] < s0, w0[i], 0) * s1 + x1[i]   (uop A)
        w2[i] = select(w1[i] < s0, w1[i], 0) * s1 + x2[i]   (uop B)

    A computes w1 in ALU blocks 0-3 (the stock 1-step program) and
    bypasses it to the write port. B (one element = one cycle behind)
    leaves blocks 0-3 idle with block 3's out-flop WRITE DISABLED, so the
    flop still holds A's w1 when B's block-4 compare reads PREV_ALU_OUT
    one cycle later; B computes in blocks 4-7. Same ALU sequence per step
    as the 1-step op -> bit-identical results, but the serial chain is
    100 instructions of 512 elements instead of 200 of 256, halving the
    per-instruction fixed cost (~151 cycles) on the critical path.

    DveOp.compile() would re-lower the placeholder Spec, so the hand
    program is pre-seeded into dve_ops._COMPILE_CACHE for both DVE
    generations; dve_table_for_ops and _custom_dve hit the cache.
    """
    import concourse.dve_ops as dve_ops
    from concourse.dve_ops import DveOp
    from concourse.dve_spec import C0, C1, Spec, Src0, Src1, Zero, select
    from concourse.dve_uop import (
        AluInp,
        AluOp,
        DelayInp,
        DveOpSpec,
        InpSel,
        OutPath,
        OutSel,
        Trigger,
        UopConfig,
    )

    if _LIF2_OP_NAME in dve_ops._SUB_OPCODE_FOR_NAME:
        for op in dve_ops.OPS:
            if op.name == _LIF2_OP_NAME:
                return op
        raise RuntimeError("LIF2 op registered but not in OPS")

    def mk_uop(first_of_pair: bool, next_idx: int) -> UopConfig:
        u = UopConfig()
        # Input lanes mirror the stock 1-step lowering: lane k feeds
        # block 0's delay chain k-1.
        u.enable_input(InpSel.SRC_1, 1)  # chain0: w0 (only A consumes it)
        u.enable_input(InpSel.CONST_0, 2)  # chain1: threshold s0
        u.enable_input(InpSel.ZERO, 3)  # chain2: 0.0
        u.enable_input(InpSel.CONST_1, 4)  # chain3: decay s1
        u.enable_input(InpSel.SRC_0, 5)  # chain4: x for this step
        u.require_inp0 = 1
        u.require_inp1 = 1
        u.repeat_count = 1
        u.trigger = (Trigger.SRC_TENSOR_DONE, Trigger.COUNT, Trigger.NONE)
        u.next_uop = (0, next_idx, 0)
        dp = u.datapath_config
        for b in range(8):
            dp[b].pass_through_delay(0, 1, 2, 3, 4)
        if first_of_pair:
            # blocks 0-3: the stock step; 4-7: bypass w1 to the write port
            dp[0].enable_alu(AluOp.IS_LT, AluInp.PREV_DELAY_0, AluInp.PREV_DELAY_1)
            dp[1].enable_alu(AluOp.SELECT, AluInp.PREV_DELAY_2, AluInp.PREV_DELAY_0)
            dp[2].enable_alu(AluOp.MULTIPLY, AluInp.PREV_ALU_OUT, AluInp.PREV_DELAY_3)
            dp[3].enable_alu(AluOp.ADD, AluInp.PREV_ALU_OUT, AluInp.PREV_DELAY_4)
            for b in (4, 5, 6, 7):
                dp[b].pass_through_alu()
        else:
            # blocks 0-3 idle (block 3's flop retains A's w1); compute in 4-7.
            dp[4].enable_alu(AluOp.IS_LT, AluInp.PREV_ALU_OUT, AluInp.PREV_DELAY_1)
            # capture w1 for the select operand (chain1: threshold is dead now)
            dp[4].enable_delay_from_src(DelayInp.PREV_ALU_OUT, 1)
            dp[5].enable_alu(AluOp.SELECT, AluInp.PREV_DELAY_2, AluInp.PREV_DELAY_1)
            dp[6].enable_alu(AluOp.MULTIPLY, AluInp.PREV_ALU_OUT, AluInp.PREV_DELAY_3)
            dp[7].enable_alu(AluOp.ADD, AluInp.PREV_ALU_OUT, AluInp.PREV_DELAY_4)
        u.enable_output(OutSel.ALU_OUT, OutPath.WR0_LO)
        return u

    # uop[0] may not be a jump target: [A-entry, B, A-loop], B <-> A-loop.
    uops = [mk_uop(True, 1), mk_uop(False, 2), mk_uop(True, 1)]

    # Placeholder Spec: gives _custom_dve the right flags (reads Src1, no
    # C2/accum) and CoreSim a 1-step reference; the hand uops are what the
    # hardware runs.
    body = select(Src0 < C0, Src0, Zero) * C1 + Src1
    spec = Spec(body=body, reference=_lif_reference)
    row = dve_ops._CUSTOM_DVE_ROW_BASE + len(dve_ops.OPS)
    handspec = DveOpSpec(name=_LIF2_OP_NAME, opcode=row, uops=uops, rd1_en=True)
    shas = {ver: handspec.sha(ver) for ver in ("v3", "v4")}
    op = DveOp(_LIF2_OP_NAME, spec, subdim=False, uops_sha=shas)
    dve_ops.OPS.append(op)
    dve_ops._SUB_OPCODE_FOR_NAME[_LIF2_OP_NAME] = row
    dve_ops.CUSTOM_DVE_SPECS[_LIF2_OP_NAME] = spec
    for ver in ("v3", "v4"):
        dve_ops._COMPILE_CACHE[(_LIF2_OP_NAME, ver)] = handspec
    return op


def _build_bass(reps: int = 1):
    # reps > 1 repeats the whole pipeline on the same buffers (benchmarking
    # only — amortizes host dispatch overhead to expose the device time).
    import concourse.bacc as bacc
    import concourse.tile as tile
    from concourse import mybir

    lif_op = _register_lif2_op() if PAIRED else _register_lif_op()

    nc = bacc.Bacc(
        "TRN2",
        target_bir_lowering=False,
        debug=False,
        enable_asserts=False,
    )

    P = P128
    f32 = mybir.dt.float32

    u8 = mybir.dt.uint8
    # Pair-interleaved DRAM layouts: [P, T/2, FREE, 2] (time pairs at the
    # innermost dim, host permutes to/from this).
    x_d = nc.dram_tensor("x", [P, T // 2, FREE, 2], f32, kind="ExternalInput").ap()
    s_d = nc.dram_tensor("spk", [P, T // 2, FREE, 2], u8, kind="ExternalOutput").ap()

    with ExitStack() as ctx:
        tc = ctx.enter_context(tile.TileContext(nc))
        xp = ctx.enter_context(tc.tile_pool(name="xp", bufs=3))
        wp = ctx.enter_context(tc.tile_pool(name="wp", bufs=2))
        sp = ctx.enter_context(tc.tile_pool(name="sp", bufs=1))
        st = ctx.enter_context(tc.tile_pool(name="st", bufs=1))

        zero = st.tile([P, FREE], f32)
        nc.vector.memset(zero[:], 0.0)
        # Spike threshold as an ACT bias: sign(w + SPIKE_BIAS) is +1 exactly
        # when w >= TH (SPIKE_BIAS = nextafter(-TH, 0), so w == TH lands one
        # ulp above zero and w == TH - 1ulp lands exactly on zero -> sign 0).
        spike_bias = st.tile([P, 1], f32, tag="bias")
        nc.vector.memset(
            spike_bias[:], float(np.nextafter(np.float32(-TH), np.float32(0)))
        )

        # Spikes accumulate in one tile per store group (deferred DMAs).
        spk_group = {}  # chunk index whose completion triggers the store -> (tile, a, b)
        for _c, (a, b) in STORE_AFTER_CHUNK.items():
            spk_group[_c] = (
                sp.tile(
                    [P, (b - a) // 2, FREE, 2], u8, name=f"spk{_c}", tag=f"s{_c}"
                ),
                a,
                b,
            )

        def group_of(t):
            for _c, (tile_, a, b) in spk_group.items():
                if a <= t < b:
                    return tile_, a, b
            raise AssertionError(t)

        wt_prev = None
        prev_tc = None
        for c, tcsz in enumerate(CHUNKS * reps):
            t0 = sum(CHUNKS[: c % len(CHUNKS)])
            xt = xp.tile([P, TCMAX // 2, FREE, 2], f32, tag="x")
            # Loads ride the SP HWDGE ring, stores the ACT HWDGE ring —
            # two independent DMA queues that overlap.
            nc.sync.dma_start(
                out=xt[:, : tcsz // 2, :, :],
                in_=x_d[:, t0 // 2 : (t0 + tcsz) // 2, :, :],
            )

            wt = wp.tile([P, TCMAX // 2, FREE, 2], f32, tag="w")
            for jp in range(tcsz // 2):
                if c == 0 and jp == 0:
                    w_in = zero[:]
                elif jp == 0:
                    w_in = wt_prev[:, prev_tc // 2 - 1, :, 1]
                else:
                    w_in = wt[:, jp - 1, :, 1]
                # Two fused steps per instruction; element stream is
                # (w1[i], w2[i]) pairs, f-major — contiguous in this
                # layout. in1 = w0 broadcast (read once per pair by uop A).
                nc.vector._custom_dve(
                    lif_op,
                    out=wt[:, jp, :, :],
                    in0=xt[:, jp, :, :],
                    in1=w_in.to_broadcast((P, FREE, 2)),
                    s0=TH,
                    s1=DECAY,
                )
            wt_prev = wt
            prev_tc = tcsz

            # spikes as sign(w + SPIKE_BIAS) in {-1, 0, +1} stored u8 (the
            # host maps ==1 -> 1.0f). Runs on the otherwise-idle Scalar
            # engine; GpSimd's tensor_scalar measures ~18 cyc/elem and
            # serializes the kernel, ACT streams at 1 elem/cycle.
            gt, ga, gb = group_of(t0)
            assert t0 + tcsz <= gb, "chunk spans store groups"
            nc.scalar.activation(
                out=gt[
                    :, (t0 - ga) // 2 : (t0 - ga + tcsz) // 2, :, :
                ].rearrange("p t f s -> p (t f s)"),
                in_=wt[:, : tcsz // 2, :, :].rearrange("p t f s -> p (t f s)"),
                func=mybir.ActivationFunctionType.Sign,
                bias=spike_bias[:],
            )
            if c % len(CHUNKS) in STORE_AFTER_CHUNK:
                a, b = STORE_AFTER_CHUNK[c % len(CHUNKS)]
                nc.scalar.dma_start(
                    out=s_d[:, a // 2 : b // 2, :, :], in_=gt[:]
                )

    # Bacc lowering: splits multi-wait instructions into event-semaphore
    # chains (TRN2 allows at most one sync wait per instruction), register
    # allocation, DCE.
    nc.compile()
    return nc


def _get_nc():
    if "nc" not in _CACHE:
        _CACHE["nc"] = _build_bass()
    return _CACHE["nc"]


def _shard_input(inputs: np.ndarray, i: int) -> np.ndarray:
    # [32, 200, 1024] -> partition-major [128, 200, 256] with p = k*32 + b,
    # then time-pair-interleaved [128, 100, 256, 2] for the fused DVE op.
    xi = inputs[i * BL : (i + 1) * BL]
    xi = xi.reshape(BL, T, NK, FREE).transpose(2, 0, 1, 3)
    xi = np.ascontiguousarray(xi).reshape(P128, T // 2, 2, FREE)
    return np.ascontiguousarray(xi.transpose(0, 1, 3, 2))


def _unshard_output(spk: np.ndarray) -> np.ndarray:
    # [128, 100, 256, 2] u8 -> un-pair -> [128, 200, 256] -> [32, 200, 1024]
    s = spk.transpose(0, 1, 3, 2).reshape(P128, T, FREE)
    s = s.reshape(NK, BL, T, FREE).transpose(1, 2, 0, 3)
    return np.ascontiguousarray(s).reshape(BL, T, N)


def kernel(inputs: np.ndarray, trace: bool = False) -> np.ndarray:
    from concourse.bass_utils import run_bass_kernel_spmd

    inputs = np.ascontiguousarray(np.asarray(inputs, dtype=np.float32))
    assert inputs.shape == (B, T, N), inputs.shape

    nc = _get_nc()
    in_maps = [{"x": _shard_input(inputs, i)} for i in range(NCORES)]
    res = run_bass_kernel_spmd(
        nc, in_maps, core_ids=list(range(NCORES)), trace=trace
    )
    _CACHE["last_results"] = res
    out = np.concatenate(
        [_unshard_output(r["spk"]) for r in res.results], axis=0
    )
    # Device stores sign(w + SPIKE_BIAS) as u8: +1 (= spike) maps to 1,
    # 0 and -1 (however the f32->u8 conversion encodes it) map to not-1.
    return (out == 1).astype(np.float32)



# revision 39
# speedup vs baseline: 1.2727x; 1.0840x over previous
"""LIF spiking-neuron scan (SimpleSNN) Trainium2 Bass kernel.

Reference semantics (per sample b, neuron n, over T timesteps):
    mem = mem * 0.9 + x[t]
    spike[t] = (mem >= 1.5)
    mem = mem * (1 - spike[t])

Full inputs [256, 200, 1024] f32 are sharded batch-wise over 8 NeuronCores
(32 samples/core; the time recurrence is per-sample so no cross-core comms).

Host-side, each core's shard [32, 200, 1024] is permuted to a
partition-major layout [128, 200, 256] with partition p = k*32 + b
(k = n // 256, b = sample), so every chunk DMA is a single dense 3-D
transfer carrying one completion semaphore.

Per-core device design:
  - The recurrence is rewritten over the PRE-reset membrane w:
        w_t = select(w_{t-1} < 1.5, w_{t-1}, 0) * 0.9 + x_t
        spike_t = (w_t >= 1.5)
    which is bit-identical to the reference (same two f32 roundings per
    step) and needs only ONE fused custom-DVE op per step (5 ALU stages
    of the DVE's 8-stage pipeline). The w history is materialized in the
    chunk tile, so the whole sequential chain is 200 back-to-back Vector
    engine instructions at ~[128, 256] each.
  - T=200 steps split into ragged chunks (geometric ramp, small tail).
    Per chunk: one HWDGE DMA load of x [128, tc, 256] (SP ring), tc
    fused LIF-step ops (DVE), one batched Sign activation over the w
    chunk on the Scalar engine (sign(w + nextafter(-1.5, 0)) is +1 u8
    exactly iff w >= 1.5; the host maps ==1 -> 1.0f).
  - Spikes stay resident in SBUF (50 KiB/partition u8) and are written
    by four large deferred DMA stores (ACT ring) scheduled behind the x
    loads, because HBM reads and writes share the ~360 GB/s per-core
    budget and interleaved stores starve the loads below the chain rate.
  - The recurrence runs as 100 two-step fused custom-DVE instructions
    (hand-written 3-uop program, see _register_lif2_op) over
    pair-interleaved tiles, ~690 ns each at nominal clock — vs 200
    one-step ops at ~425 ns — making the x loads (26.2 MB at the ~360
    GB/s per-core HBM cap) the critical path.
  - Measured: 110.2 us/core on a ~20%-clock-throttled sample, ~92-97 us
    expected at nominal DVE clock (vs 801.4 us baseline). ~12 us is
    fixed startup (engine barriers + table loads + first chunk load),
    ~5 us tail (last Sign + store + final barrier).
"""

from contextlib import ExitStack

import numpy as np

B, T, N = 256, 200, 1024
NCORES = 8
BL = B // NCORES  # 32 samples per core
DECAY = 0.9
TH = 1.5
P128 = 128
FREE = 256  # free-dim size of the state tile
NK = N // FREE  # 4 n-blocks; partition p = k*32 + b
# Ragged chunking. The x loads (131 kB/step) at the ~360 GB/s per-core
# HBM cap run only ~20% faster than the serial DVE chain (~0.46
# us/step), so the loader builds slack slowly: start with a small chunk
# (chain starts ~3 us after the first bytes land) and grow
# geometrically. Small last chunk keeps the tail (final spike pass +
# store after the chain ends) short.
# The serial chain uses the hand-written two-step fused DVE op (100
# instructions instead of 200, halving the ~151-cycle per-instruction
# fixed cost on the critical path). The x/w/spike layouts are
# PAIR-INTERLEAVED [P, T/2, FREE, 2] so the fused op's in0/out streams
# are unit-stride: with the plain layout the pair APs jumped 1 KiB at
# the innermost step and the SBUF port's 8-byte fetch granularity
# dropped the stream to ~2 cycles/element (1070 ns/op measured vs the
# ~690 ns contiguous cost).
PAIRED = True
# All chunk sizes even: the fused DVE op processes time-step PAIRS.
# With the fused chain (~345 ns/step) outrunning the x loads (~390
# ns/step at the HBM cap), the loader - not the chain - is the critical
# path: chunk sizes only need a small first chunk (fast chain start),
# big middles (DMA efficiency, few boundaries), and small tail chunks
# (short post-chain tail).
CHUNKS = [4, 8, 12, 20, 24, 24, 24, 24, 24, 18, 10, 4, 4]
assert sum(CHUNKS) == T
TCMAX = max(CHUNKS)
# Spikes for all 200 steps stay resident in SBUF (T*FREE u8 = 50 KiB per
# partition) and are stored in a few large deferred DMAs, scheduled so
# the writes drain mostly after the loads finish: HBM read+write share
# the ~360 GB/s per-core budget, and interleaved per-chunk stores were
# measured to slow the loads below the chain rate (pipeline stalls).
# Each store group gets its OWN SBUF tile: a single shared tile made
# Tile's whole-tile WAR tracking stall later Sign ops behind earlier
# groups' store reads (measured 6.8 us chain stall). The shrinking tail
# chunks keep the post-chain work (last Sign + last store) tiny.
STORE_AFTER_CHUNK = {7: (0, 140), 9: (140, 182), 11: (182, 196), 12: (196, 200)}
for _c, (_a, _b) in STORE_AFTER_CHUNK.items():
    assert sum(CHUNKS[: _c + 1]) == _b

_CACHE = {}

_LIF_OP_NAME = "LIF_STEP_ANT"


def _lif_reference(in0, in1, s0, s1, imm2):
    return (
        np.where(in0 < np.float32(s0), in0, np.float32(0.0)) * np.float32(s1) + in1
    ).astype(np.float32)


def _register_lif_op():
    """Register the fused LIF-step custom DVE op:
        out = select(in0 < s0, in0, 0) * s1 + in1
    (in0 = previous membrane w, in1 = x_t, s0 = threshold, s1 = decay).
    Registration is the runtime equivalent of appending to dve_ops.OPS;
    uops_sha is computed from the same lower() used at compile time.
    """
    import concourse.dve_ops as dve_ops
    from concourse.dve_ops import DveOp
    from concourse.dve_spec import C0, C1, Spec, Src0, Src1, Zero, lower, select
    from concourse.dve_uop import DveOpSpec

    if _LIF_OP_NAME in dve_ops._SUB_OPCODE_FOR_NAME:
        for op in dve_ops.OPS:
            if op.name == _LIF_OP_NAME:
                return op
        raise RuntimeError("LIF op registered but not in OPS")

    body = select(Src0 < C0, Src0, Zero) * C1 + Src1
    spec = Spec(body=body, reference=_lif_reference)
    row = dve_ops._CUSTOM_DVE_ROW_BASE + len(dve_ops.OPS)
    shas = {}
    for ver in ("v3", "v4"):
        uops = lower(spec, ver=ver)
        shas[ver] = DveOpSpec(
            name=_LIF_OP_NAME, opcode=row, uops=uops, rd1_en=True
        ).sha(ver)
    op = DveOp(_LIF_OP_NAME, spec, subdim=False, uops_sha=shas)
    dve_ops.OPS.append(op)
    dve_ops._SUB_OPCODE_FOR_NAME[_LIF_OP_NAME] = row
    dve_ops.CUSTOM_DVE_SPECS[_LIF_OP_NAME] = spec
    return op


_LIF2_OP_NAME = "LIF_STEP2_ANT"


def _register_lif2_op():
    """Register a hand-written two-step fused LIF op. One instruction
    processes PAIRS of time steps: the element stream is (w1[i], w2[i])
    pairs over i (f-major, pair-minor APs on the normal tile layout):

        w1[i] = select(w0[i] < s0, w0[i], 0) * s1 + x1[i]   (uop A)
        w2[i] = select(w1[i] < s0, w1[i], 0) * s1 + x2[i]   (uop B)

    A computes w1 in ALU blocks 0-3 (the stock 1-step program) and
    bypasses it to the write port. B (one element = one cycle behind)
    leaves blocks 0-3 idle with block 3's out-flop WRITE DISABLED, so the
    flop still holds A's w1 when B's block-4 compare reads PREV_ALU_OUT
    one cycle later; B computes in blocks 4-7. Same ALU sequence per step
    as the 1-step op -> bit-identical results, but the serial chain is
    100 instructions of 512 elements instead of 200 of 256, halving the
    per-instruction fixed cost (~151 cycles) on the critical path.

    DveOp.compile() would re-lower the placeholder Spec, so the hand
    program is pre-seeded into dve_ops._COMPILE_CACHE for both DVE
    generations; dve_table_for_ops and _custom_dve hit the cache.
    """
    import concourse.dve_ops as dve_ops
    from concourse.dve_ops import DveOp
    from concourse.dve_spec import C0, C1, Spec, Src0, Src1, Zero, select
    from concourse.dve_uop import (
        AluInp,
        AluOp,
        DelayInp,
        DveOpSpec,
        InpSel,
        OutPath,
        OutSel,
        Trigger,
        UopConfig,
    )

    if _LIF2_OP_NAME in dve_ops._SUB_OPCODE_FOR_NAME:
        for op in dve_ops.OPS:
            if op.name == _LIF2_OP_NAME:
                return op
        raise RuntimeError("LIF2 op registered but not in OPS")

    def mk_uop(first_of_pair: bool, next_idx: int) -> UopConfig:
        u = UopConfig()
        # Input lanes mirror the stock 1-step lowering: lane k feeds
        # block 0's delay chain k-1.
        u.enable_input(InpSel.SRC_1, 1)  # chain0: w0 (only A consumes it)
        u.enable_input(InpSel.CONST_0, 2)  # chain1: threshold s0
        u.enable_input(InpSel.ZERO, 3)  # chain2: 0.0
        u.enable_input(InpSel.CONST_1, 4)  # chain3: decay s1
        u.enable_input(InpSel.SRC_0, 5)  # chain4: x for this step
        u.require_inp0 = 1
        u.require_inp1 = 1
        u.repeat_count = 1
        u.trigger = (Trigger.SRC_TENSOR_DONE, Trigger.COUNT, Trigger.NONE)
        u.next_uop = (0, next_idx, 0)
        dp = u.datapath_config
        for b in range(8):
            dp[b].pass_through_delay(0, 1, 2, 3, 4)
        if first_of_pair:
            # blocks 0-3: the stock step; 4-7: bypass w1 to the write port
            dp[0].enable_alu(AluOp.IS_LT, AluInp.PREV_DELAY_0, AluInp.PREV_DELAY_1)
            dp[1].enable_alu(AluOp.SELECT, AluInp.PREV_DELAY_2, AluInp.PREV_DELAY_0)
            dp[2].enable_alu(AluOp.MULTIPLY, AluInp.PREV_ALU_OUT, AluInp.PREV_DELAY_3)
            dp[3].enable_alu(AluOp.ADD, AluInp.PREV_ALU_OUT, AluInp.PREV_DELAY_4)
            for b in (4, 5, 6, 7):
                dp[b].pass_through_alu()
        else:
            # blocks 0-3 idle (block 3's flop retains A's w1); compute in 4-7.
            dp[4].enable_alu(AluOp.IS_LT, AluInp.PREV_ALU_OUT, AluInp.PREV_DELAY_1)
            # capture w1 for the select operand (chain1: threshold is dead now)
            dp[4].enable_delay_from_src(DelayInp.PREV_ALU_OUT, 1)
            dp[5].enable_alu(AluOp.SELECT, AluInp.PREV_DELAY_2, AluInp.PREV_DELAY_1)
            dp[6].enable_alu(AluOp.MULTIPLY, AluInp.PREV_ALU_OUT, AluInp.PREV_DELAY_3)
            dp[7].enable_alu(AluOp.ADD, AluInp.PREV_ALU_OUT, AluInp.PREV_DELAY_4)
        u.enable_output(OutSel.ALU_OUT, OutPath.WR0_LO)
        return u

    # uop[0] may not be a jump target: [A-entry, B, A-loop], B <-> A-loop.
    uops = [mk_uop(True, 1), mk_uop(False, 2), mk_uop(True, 1)]

    # Placeholder Spec: gives _custom_dve the right flags (reads Src1, no
    # C2/accum) and CoreSim a 1-step reference; the hand uops are what the
    # hardware runs.
    body = select(Src0 < C0, Src0, Zero) * C1 + Src1
    spec = Spec(body=body, reference=_lif_reference)
    row = dve_ops._CUSTOM_DVE_ROW_BASE + len(dve_ops.OPS)
    handspec = DveOpSpec(name=_LIF2_OP_NAME, opcode=row, uops=uops, rd1_en=True)
    shas = {ver: handspec.sha(ver) for ver in ("v3", "v4")}
    op = DveOp(_LIF2_OP_NAME, spec, subdim=False, uops_sha=shas)
    dve_ops.OPS.append(op)
    dve_ops._SUB_OPCODE_FOR_NAME[_LIF2_OP_NAME] = row
    dve_ops.CUSTOM_DVE_SPECS[_LIF2_OP_NAME] = spec
    for ver in ("v3", "v4"):
        dve_ops._COMPILE_CACHE[(_LIF2_OP_NAME, ver)] = handspec
    return op


def _build_bass(reps: int = 1):
    # reps > 1 repeats the whole pipeline on the same buffers (benchmarking
    # only — amortizes host dispatch overhead to expose the device time).
    import concourse.bacc as bacc
    import concourse.tile as tile
    from concourse import mybir

    lif_op = _register_lif2_op() if PAIRED else _register_lif_op()

    nc = bacc.Bacc(
        "TRN2",
        target_bir_lowering=False,
        debug=False,
        enable_asserts=False,
    )

    P = P128
    f32 = mybir.dt.float32

    u8 = mybir.dt.uint8
    # Pair-interleaved DRAM layouts: [P, T/2, FREE, 2] (time pairs at the
    # innermost dim, host permutes to/from this).
    x_d = nc.dram_tensor("x", [P, T // 2, FREE, 2], f32, kind="ExternalInput").ap()
    s_d = nc.dram_tensor("spk", [P, T // 2, FREE, 2], u8, kind="ExternalOutput").ap()

    with ExitStack() as ctx:
        tc = ctx.enter_context(tile.TileContext(nc))
        xp = ctx.enter_context(tc.tile_pool(name="xp", bufs=3))
        wp = ctx.enter_context(tc.tile_pool(name="wp", bufs=2))
        sp = ctx.enter_context(tc.tile_pool(name="sp", bufs=1))
        st = ctx.enter_context(tc.tile_pool(name="st", bufs=1))

        zero = st.tile([P, FREE], f32)
        nc.vector.memset(zero[:], 0.0)
        # Spike threshold as an ACT bias: sign(w + SPIKE_BIAS) is +1 exactly
        # when w >= TH (SPIKE_BIAS = nextafter(-TH, 0), so w == TH lands one
        # ulp above zero and w == TH - 1ulp lands exactly on zero -> sign 0).
        spike_bias = st.tile([P, 1], f32, tag="bias")
        nc.vector.memset(
            spike_bias[:], float(np.nextafter(np.float32(-TH), np.float32(0)))
        )

        # Spikes accumulate in one tile per store group (deferred DMAs).
        spk_group = {}  # chunk index whose completion triggers the store -> (tile, a, b)
        for _c, (a, b) in STORE_AFTER_CHUNK.items():
            spk_group[_c] = (
                sp.tile(
                    [P, (b - a) // 2, FREE, 2], u8, name=f"spk{_c}", tag=f"s{_c}"
                ),
                a,
                b,
            )

        def group_of(t):
            for _c, (tile_, a, b) in spk_group.items():
                if a <= t < b:
                    return tile_, a, b
            raise AssertionError(t)

        wt_prev = None
        prev_tc = None
        for c, tcsz in enumerate(CHUNKS * reps):
            t0 = sum(CHUNKS[: c % len(CHUNKS)])
            xt = xp.tile([P, TCMAX // 2, FREE, 2], f32, tag="x")
            # Loads ride the SP HWDGE ring, stores the ACT HWDGE ring —
            # two independent DMA queues that overlap.
            nc.sync.dma_start(
                out=xt[:, : tcsz // 2, :, :],
                in_=x_d[:, t0 // 2 : (t0 + tcsz) // 2, :, :],
            )

            wt = wp.tile([P, TCMAX // 2, FREE, 2], f32, tag="w")
            for jp in range(tcsz // 2):
                if c == 0 and jp == 0:
                    w_in = zero[:]
                elif jp == 0:
                    w_in = wt_prev[:, prev_tc // 2 - 1, :, 1]
                else:
                    w_in = wt[:, jp - 1, :, 1]
                # Two fused steps per instruction; element stream is
                # (w1[i], w2[i]) pairs, f-major — contiguous in this
                # layout. in1 = w0 broadcast (read once per pair by uop A).
                nc.vector._custom_dve(
                    lif_op,
                    out=wt[:, jp, :, :],
                    in0=xt[:, jp, :, :],
                    in1=w_in.to_broadcast((P, FREE, 2)),
                    s0=TH,
                    s1=DECAY,
                )
            wt_prev = wt
            prev_tc = tcsz

            # spikes as sign(w + SPIKE_BIAS) in {-1, 0, +1} stored u8 (the
            # host maps ==1 -> 1.0f). Runs on the otherwise-idle Scalar
            # engine; GpSimd's tensor_scalar measures ~18 cyc/elem and
            # serializes the kernel, ACT streams at 1 elem/cycle.
            gt, ga, gb = group_of(t0)
            assert t0 + tcsz <= gb, "chunk spans store groups"
            nc.scalar.activation(
                out=gt[
                    :, (t0 - ga) // 2 : (t0 - ga + tcsz) // 2, :, :
                ].rearrange("p t f s -> p (t f s)"),
                in_=wt[:, : tcsz // 2, :, :].rearrange("p t f s -> p (t f s)"),
                func=mybir.ActivationFunctionType.Sign,
                bias=spike_bias[:],
            )
            if c % len(CHUNKS) in STORE_AFTER_CHUNK:
                a, b = STORE_AFTER_CHUNK[c % len(CHUNKS)]
                nc.scalar.dma_start(
                    out=s_d[:, a // 2 : b // 2, :, :], in_=gt[:]
                )

    # Bacc lowering: splits multi-wait instructions into event-semaphore
    # chains (TRN2 allows at most one sync wait per instruction), register
    # allocation, DCE.
    nc.compile()
    return nc


def _get_nc():
    if "nc" not in _CACHE:
        _CACHE["nc"] = _build_bass()
    return _CACHE["nc"]


def _shard_input(inputs: np.ndarray, i: int) -> np.ndarray:
    # [32, 200, 1024] -> partition-major [128, 200, 256] with p = k*32 + b,
    # then time-pair-interleaved [128, 100, 256, 2] for the fused DVE op.
    xi = inputs[i * BL : (i + 1) * BL]
    xi = xi.reshape(BL, T, NK, FREE).transpose(2, 0, 1, 3)
    xi = np.ascontiguousarray(xi).reshape(P128, T // 2, 2, FREE)
    return np.ascontiguousarray(xi.transpose(0, 1, 3, 2))


def _unshard_output(spk: np.ndarray) -> np.ndarray:
    # [128, 100, 256, 2] u8 -> un-pair -> [128, 200, 256] -> [32, 200, 1024]
    s = spk.transpose(0, 1, 3, 2).reshape(P128, T, FREE)
    s = s.reshape(NK, BL, T, FREE).transpose(1, 2, 0, 3)
    return np.ascontiguousarray(s).reshape(BL, T, N)


def kernel(inputs: np.ndarray, trace: bool = False) -> np.ndarray:
    from concourse.bass_utils import run_bass_kernel_spmd

    inputs = np.ascontiguousarray(np.asarray(inputs, dtype=np.float32))
    assert inputs.shape == (B, T, N), inputs.shape

    nc = _get_nc()
    in_maps = [{"x": _shard_input(inputs, i)} for i in range(NCORES)]
    res = run_bass_kernel_spmd(
        nc, in_maps, core_ids=list(range(NCORES)), trace=trace
    )
    _CACHE["last_results"] = res
    out = np.concatenate(
        [_unshard_output(r["spk"]) for r in res.results], axis=0
    )
    # Device stores sign(w + SPIKE_BIAS) as u8: +1 (= spike) maps to 1,
    # 0 and -1 (however the f32->u8 conversion encodes it) map to not-1.
    return (out == 1).astype(np.float32)

